# revision 12
# baseline (speedup 1.0000x reference)
"""Trainium2 Bass kernel for capsule-network AgreementRouting (n_iterations=1).

Reference computation (see problem):
    c = softmax(b, axis=-1)                  # [IN, OUT] (same for every batch)
    s[b,o,d] = sum_in c[in,o] * u[b,in,o,d]  # weighted reduce over input caps
    v = squash(s)                            # per (b,o): s * l2/(1+l2)/sqrt(l2)
    out = v[:, None]                         # [B, 1, OUT, DIM]

Strategy: data-parallel over batch across 8 NeuronCores (64 batches/core).
Per core the 47 MB u-shard is streamed through SBUF in 3-batch groups with
contiguous 5760B-per-partition DMAs; each group's in-caps reduction runs on
TensorE with softmax(b) chunks as the stationary operand
(out[o', (g,o,d)] = sum_in c[in,o'] u[in,g,o,d]), the o'==o diagonal is
extracted with a precomputed identity mask + strided reduce on VectorE, and
squash + store follow immediately per group so the post-stream tail is just
the final (2-batch) group's chain.
"""

import numpy as np

import concourse.bass as bass
import concourse.tile as tile
from concourse import bacc, mybir
from concourse.bass_utils import run_bass_kernel_spmd

F32 = mybir.dt.float32
F32R = mybir.dt.float32r

B, IN_CAPS, OUT_CAPS, OUT_DIM = 512, 1152, 10, 16
N_CORES = 8
B_LOCAL = B // N_CORES            # 64 batches per core
OD = OUT_CAPS * OUT_DIM           # 160
P = 128                           # partitions
N_CHUNKS = IN_CAPS // P           # 9 contraction chunks
GROUP = 3                         # batches per PSUM accumulation group (480 cols)

# DMA plan: 10 six-batch tiles (two 3-batch groups each) for the bulk of the
# stream, then two 2-batch tiles whose DMAs are split by chunk range so the
# final groups' matmuls overlap their own loads — keeps the post-stream tail
# down to one short dependent chain.
TILE_B = 6
DMA_PLAN = [(i * TILE_B, TILE_B) for i in range(10)] + [(60, 2), (62, 2)]
N_SPLIT = 6                       # chunk split point for the final DMA


def _build_core_program() -> bass.Bass:
    nc = bacc.Bacc(None)
    u = nc.dram_tensor("u", [B_LOCAL, IN_CAPS, OUT_CAPS, OUT_DIM], F32,
                       kind="ExternalInput")
    bp = nc.dram_tensor("b", [IN_CAPS, OUT_CAPS], F32, kind="ExternalInput")
    v = nc.dram_tensor("v", [OUT_CAPS, B_LOCAL, OUT_DIM], F32,
                       kind="ExternalOutput")

    # in-cap index mapping: in = p*N_CHUNKS + n (partition-major). Per (p, b)
    # the 9 chunk rows are contiguous in HBM -> 5760B runs per partition for
    # u and a single 360B run for b, keeping each DMA on one descriptor lane.
    u_r = u[:].rearrange("b (p n) o d -> p b n (o d)", p=P)
    b_r = bp[:].rearrange("(p n) o -> p n o", p=P)

    with tile.TileContext(nc) as tc:
        with (
            tc.tile_pool(name="singles", bufs=1) as singles,
            tc.tile_pool(name="inp", bufs=5) as inp,
            tc.tile_pool(name="psum", bufs=8, space="PSUM") as psum,
            tc.tile_pool(name="mids", bufs=4) as mids,
            tc.tile_pool(name="sq", bufs=4) as sqp,
        ):
            # ---- softmax over b rows: c[in, o] ----
            b_sb = singles.tile([P, N_CHUNKS, OUT_CAPS], F32)
            nc.scalar.dma_start(out=b_sb, in_=b_r)
            bmax = singles.tile([P, N_CHUNKS], F32)
            nc.vector.reduce_max(out=bmax, in_=b_sb, axis=mybir.AxisListType.X)
            negmax = singles.tile([P, N_CHUNKS], F32)
            nc.scalar.mul(out=negmax, in_=bmax, mul=-1.0)
            e_sb = singles.tile([P, N_CHUNKS, OUT_CAPS], F32)
            for n in range(N_CHUNKS):
                nc.scalar.activation(
                    out=e_sb[:, n, :], in_=b_sb[:, n, :],
                    func=mybir.ActivationFunctionType.Exp,
                    bias=negmax[:, n : n + 1], scale=1.0,
                )
            esum = singles.tile([P, N_CHUNKS], F32)
            nc.vector.reduce_sum(out=esum, in_=e_sb, axis=mybir.AxisListType.X)
            einv = singles.tile([P, N_CHUNKS], F32)
            nc.vector.reciprocal(out=einv, in_=esum)
            c_sb = singles.tile([P, N_CHUNKS, OUT_CAPS], F32R)
            for n in range(N_CHUNKS):
                nc.vector.tensor_scalar_mul(
                    out=c_sb[:, n, :], in0=e_sb[:, n, :],
                    scalar1=einv[:, n : n + 1],
                )

            # ---- tile-0 u load on the gpsimd SWDGE ring: its prebuilt
            # descriptors flood the DMA engines before the SP HWDGE ring
            # finishes generating tile 1's ----
            u_t0 = inp.tile([P, TILE_B, N_CHUNKS, OD], F32R,
                            tag="u_t", name="u_t")
            nc.gpsimd.dma_start(
                out=u_t0[:, :TILE_B],
                in_=u_r[:, 0:TILE_B].bitcast(F32R),
            )

            # ---- diagonal-selection mask: mask[o', g, o, d] = (o == o') ----
            mask = singles.tile([OUT_CAPS, GROUP, OUT_CAPS, OUT_DIM], F32)
            nc.gpsimd.memset(mask, 0.0)
            nc.gpsimd.affine_select(
                out=mask, in_=mask,
                compare_op=mybir.AluOpType.not_equal,
                fill=1.0, base=0, channel_multiplier=1,
                pattern=[[0, GROUP], [-1, OUT_CAPS], [0, OUT_DIM]],
            )

            def matmuls(ps, u_t, g0: int, gs: int):
                for n in range(N_CHUNKS):
                    # float32r: fp32 bits, single-pass (tf32-like) matmul
                    nc.tensor.matmul(
                        ps,
                        c_sb[:, n, :],
                        u_t[:, g0 : g0 + gs, n, :],
                        start=(n == 0), stop=(n == N_CHUNKS - 1),
                    )

            def stage_a(ps, gs: int):
                """Diagonal extract + length stats. Vector-only except the
                sqrt, which is issued to the idle Scalar engine so the
                vector queue never blocks on it here."""
                masked = mids.tile(
                    [OUT_CAPS, GROUP, OUT_CAPS, OUT_DIM], F32,
                    tag="masked", name="masked")[:, :gs]
                nc.vector.tensor_mul(
                    out=masked,
                    in0=ps.rearrange("q g (o d) -> q g o d", d=OUT_DIM),
                    in1=mask[:, :gs],
                )
                s_g = sqp.tile([OUT_CAPS, GROUP, OUT_DIM], F32,
                               tag="s_g", name="s_g")[:, :gs]
                nc.vector.reduce_sum(
                    out=s_g,
                    in_=masked.rearrange("q g o d -> q g d o"),
                    axis=mybir.AxisListType.X,
                )
                sq = sqp.tile([OUT_CAPS, GROUP, OUT_DIM], F32,
                              tag="sq", name="sq")[:, :gs]
                nc.vector.tensor_mul(out=sq, in0=s_g, in1=s_g)
                l2 = sqp.tile([OUT_CAPS, GROUP], F32, tag="l2", name="l2")[:, :gs]
                nc.vector.reduce_sum(out=l2, in_=sq, axis=mybir.AxisListType.X)
                rt = sqp.tile([OUT_CAPS, GROUP], F32, tag="rt", name="rt")[:, :gs]
                nc.scalar.sqrt(out=rt, in_=l2)
                denom = sqp.tile([OUT_CAPS, GROUP], F32,
                                 tag="denom", name="denom")[:, :gs]
                nc.vector.tensor_scalar_add(out=denom, in0=l2, scalar1=1.0)
                dinv = sqp.tile([OUT_CAPS, GROUP], F32,
                                tag="dinv", name="dinv")[:, :gs]
                nc.vector.reciprocal(out=dinv, in_=denom)
                return s_g, rt, dinv

            def stage_b(st, b0: int, gs: int, ring):
                """Finish squash scale and store the group."""
                s_g, rt, dinv = st
                scl = sqp.tile([OUT_CAPS, GROUP], F32, tag="scl", name="scl")[:, :gs]
                nc.vector.tensor_mul(out=scl, in0=rt, in1=dinv)
                # broadcast scl over d via a stride-0 AP
                scl_b = bass.AP(
                    tensor=scl.tensor, offset=scl.offset,
                    ap=[scl.ap[0], [scl.ap[1][0], gs], [0, OUT_DIM]],
                )
                v_g = sqp.tile([OUT_CAPS, GROUP, OUT_DIM], F32,
                               tag="v_g", name="v_g")[:, :gs]
                nc.vector.tensor_mul(out=v_g, in0=s_g, in1=scl_b)
                ring.dma_start(out=v[:, b0 : b0 + gs, :], in_=v_g)

            # ---- main streaming loop (stage-B pipelined one group back) ----
            pending = None
            for ti, (tb0, tb) in enumerate(DMA_PLAN):
                last_tile = ti == len(DMA_PLAN) - 1
                if ti == 0:
                    u_t = u_t0
                else:
                    u_t = inp.tile([P, TILE_B, N_CHUNKS, OD], F32R,
                                   tag="u_t", name="u_t")
                if ti == 0:
                    pass
                elif not last_tile:
                    nc.sync.dma_start(
                        out=u_t[:, :tb],
                        in_=u_r[:, tb0 : tb0 + tb].bitcast(F32R),
                    )
                else:
                    # final tile: split by chunk range so the first matmuls
                    # overlap the remainder of this tile's own load
                    nc.sync.dma_start(
                        out=u_t[:, :tb, :N_SPLIT],
                        in_=u_r[:, tb0 : tb0 + tb, :N_SPLIT].bitcast(F32R),
                    )
                    nc.sync.dma_start(
                        out=u_t[:, :tb, N_SPLIT:],
                        in_=u_r[:, tb0 : tb0 + tb, N_SPLIT:].bitcast(F32R),
                    )
                g0 = 0
                while g0 < tb:
                    gs = min(GROUP, tb - g0)
                    ps = psum.tile([OUT_CAPS, GROUP, OD], F32,
                                   tag="ps", name="ps")[:, :gs]
                    matmuls(ps, u_t, g0, gs)
                    st = stage_a(ps, gs)
                    if pending is not None:
                        stage_b(*pending)
                    last_group = last_tile and g0 + gs >= tb
                    # the final store rides the SP ring, idle once u loads end
                    pending = (st, tb0 + g0,
                               gs, nc.sync if last_group else nc.scalar)
                    g0 += gs
            stage_b(*pending)

    nc.compile()
    return nc


_NC_CACHE = None


def _get_program() -> bass.Bass:
    global _NC_CACHE
    if _NC_CACHE is None:
        _NC_CACHE = _build_core_program()
    return _NC_CACHE


def kernel(u_predict: np.ndarray, b: np.ndarray, n_iterations) -> np.ndarray:
    u_predict = np.ascontiguousarray(np.asarray(u_predict, dtype=np.float32))
    b = np.ascontiguousarray(np.asarray(b, dtype=np.float32))
    nc = _get_program()
    in_maps = [
        {"u": u_predict[i * B_LOCAL : (i + 1) * B_LOCAL], "b": b}
        for i in range(N_CORES)
    ]
    results = run_bass_kernel_spmd(nc, in_maps, list(range(N_CORES))).results
    # per-core v is [OUT_CAPS, B_LOCAL, OUT_DIM] -> assemble [B, OUT, DIM]
    vs = np.stack([results[i]["v"] for i in range(N_CORES)])
    out = vs.transpose(0, 2, 1, 3).reshape(B, OUT_CAPS, OUT_DIM)
    if int(n_iterations) >= 1:
        out = out[:, None]
    return np.ascontiguousarray(out.astype(np.float32))


# revision 13
# speedup vs baseline: 1.0394x; 1.0394x over previous
"""Trainium2 Bass kernel for capsule-network AgreementRouting (n_iterations=1).

Reference computation (see problem):
    c = softmax(b, axis=-1)                  # [IN, OUT] (same for every batch)
    s[b,o,d] = sum_in c[in,o] * u[b,in,o,d]  # weighted reduce over input caps
    v = squash(s)                            # per (b,o): s * l2/(1+l2)/sqrt(l2)
    out = v[:, None]                         # [B, 1, OUT, DIM]

Strategy: data-parallel over batch across 8 NeuronCores (64 batches/core).
Per core the 47 MB u-shard is streamed through SBUF in 3-batch groups with
contiguous 5760B-per-partition DMAs; each group's in-caps reduction runs on
TensorE with softmax(b) chunks as the stationary operand
(out[o', (g,o,d)] = sum_in c[in,o'] u[in,g,o,d]), the o'==o diagonal is
extracted with a precomputed identity mask + strided reduce on VectorE, and
squash + store follow immediately per group so the post-stream tail is just
the final (2-batch) group's chain.
"""

import numpy as np

import concourse.bass as bass
import concourse.tile as tile
from concourse import bacc, mybir
from concourse.bass_utils import run_bass_kernel_spmd

F32 = mybir.dt.float32
F32R = mybir.dt.float32r

B, IN_CAPS, OUT_CAPS, OUT_DIM = 512, 1152, 10, 16
N_CORES = 8
B_LOCAL = B // N_CORES            # 64 batches per core
OD = OUT_CAPS * OUT_DIM           # 160
P = 128                           # partitions
N_CHUNKS = IN_CAPS // P           # 9 contraction chunks
GROUP = 3                         # batches per PSUM accumulation group (480 cols)

# DMA plan: 10 six-batch tiles (two 3-batch groups each) for the bulk of the
# stream, then two 2-batch tiles whose DMAs are split by chunk range so the
# final groups' matmuls overlap their own loads — keeps the post-stream tail
# down to one short dependent chain.
TILE_B = 6
DMA_PLAN = [(i * TILE_B, TILE_B) for i in range(10)] + [(60, 2), (62, 2)]
N_SPLIT = 6                       # chunk split point for the final DMA


def _build_core_program() -> bass.Bass:
    nc = bacc.Bacc(None)
    u = nc.dram_tensor("u", [B_LOCAL, IN_CAPS, OUT_CAPS, OUT_DIM], F32,
                       kind="ExternalInput")
    bp = nc.dram_tensor("b", [IN_CAPS, OUT_CAPS], F32, kind="ExternalInput")
    v = nc.dram_tensor("v", [OUT_CAPS, B_LOCAL, OUT_DIM], F32,
                       kind="ExternalOutput")

    # in-cap index mapping: in = p*N_CHUNKS + n (partition-major). Per (p, b)
    # the 9 chunk rows are contiguous in HBM -> 5760B runs per partition for
    # u and a single 360B run for b, keeping each DMA on one descriptor lane.
    u_r = u[:].rearrange("b (p n) o d -> p b n (o d)", p=P)
    b_r = bp[:].rearrange("(p n) o -> p n o", p=P)

    with tile.TileContext(nc) as tc:
        with (
            tc.tile_pool(name="singles", bufs=1) as singles,
            tc.tile_pool(name="inp", bufs=5) as inp,
            tc.tile_pool(name="psum", bufs=8, space="PSUM") as psum,
            tc.tile_pool(name="mids", bufs=4) as mids,
            tc.tile_pool(name="sq", bufs=4) as sqp,
        ):
            # ---- softmax over b rows: c[in, o] ----
            b_sb = singles.tile([P, N_CHUNKS, OUT_CAPS], F32)
            nc.scalar.dma_start(out=b_sb, in_=b_r)
            bmax = singles.tile([P, N_CHUNKS], F32)
            nc.vector.reduce_max(out=bmax, in_=b_sb, axis=mybir.AxisListType.X)
            negmax = singles.tile([P, N_CHUNKS], F32)
            nc.scalar.mul(out=negmax, in_=bmax, mul=-1.0)
            e_sb = singles.tile([P, N_CHUNKS, OUT_CAPS], F32)
            for n in range(N_CHUNKS):
                nc.scalar.activation(
                    out=e_sb[:, n, :], in_=b_sb[:, n, :],
                    func=mybir.ActivationFunctionType.Exp,
                    bias=negmax[:, n : n + 1], scale=1.0,
                )
            esum = singles.tile([P, N_CHUNKS], F32)
            nc.vector.reduce_sum(out=esum, in_=e_sb, axis=mybir.AxisListType.X)
            einv = singles.tile([P, N_CHUNKS], F32)
            nc.vector.reciprocal(out=einv, in_=esum)
            c_sb = singles.tile([P, N_CHUNKS, OUT_CAPS], F32R)
            for n in range(N_CHUNKS):
                nc.vector.tensor_scalar_mul(
                    out=c_sb[:, n, :], in0=e_sb[:, n, :],
                    scalar1=einv[:, n : n + 1],
                )

            # ---- diagonal-selection mask: mask[o', g, o, d] = (o == o') ----
            mask = singles.tile([OUT_CAPS, GROUP, OUT_CAPS, OUT_DIM], F32)
            nc.gpsimd.memset(mask, 0.0)
            nc.gpsimd.affine_select(
                out=mask, in_=mask,
                compare_op=mybir.AluOpType.not_equal,
                fill=1.0, base=0, channel_multiplier=1,
                pattern=[[0, GROUP], [-1, OUT_CAPS], [0, OUT_DIM]],
            )

            def matmuls(ps, u_t, g0: int, gs: int):
                for n in range(N_CHUNKS):
                    # float32r: fp32 bits, single-pass (tf32-like) matmul
                    nc.tensor.matmul(
                        ps,
                        c_sb[:, n, :],
                        u_t[:, g0 : g0 + gs, n, :],
                        start=(n == 0), stop=(n == N_CHUNKS - 1),
                    )

            def stage_a(ps, gs: int):
                """Diagonal extract + length stats. Vector-only except the
                sqrt, which is issued to the idle Scalar engine so the
                vector queue never blocks on it here."""
                masked = mids.tile(
                    [OUT_CAPS, GROUP, OUT_CAPS, OUT_DIM], F32,
                    tag="masked", name="masked")[:, :gs]
                nc.vector.tensor_mul(
                    out=masked,
                    in0=ps.rearrange("q g (o d) -> q g o d", d=OUT_DIM),
                    in1=mask[:, :gs],
                )
                s_g = sqp.tile([OUT_CAPS, GROUP, OUT_DIM], F32,
                               tag="s_g", name="s_g")[:, :gs]
                nc.vector.reduce_sum(
                    out=s_g,
                    in_=masked.rearrange("q g o d -> q g d o"),
                    axis=mybir.AxisListType.X,
                )
                sq = sqp.tile([OUT_CAPS, GROUP, OUT_DIM], F32,
                              tag="sq", name="sq")[:, :gs]
                nc.vector.tensor_mul(out=sq, in0=s_g, in1=s_g)
                l2 = sqp.tile([OUT_CAPS, GROUP], F32, tag="l2", name="l2")[:, :gs]
                nc.vector.reduce_sum(out=l2, in_=sq, axis=mybir.AxisListType.X)
                rt = sqp.tile([OUT_CAPS, GROUP], F32, tag="rt", name="rt")[:, :gs]
                nc.scalar.sqrt(out=rt, in_=l2)
                denom = sqp.tile([OUT_CAPS, GROUP], F32,
                                 tag="denom", name="denom")[:, :gs]
                nc.vector.tensor_scalar_add(out=denom, in0=l2, scalar1=1.0)
                dinv = sqp.tile([OUT_CAPS, GROUP], F32,
                                tag="dinv", name="dinv")[:, :gs]
                nc.vector.reciprocal(out=dinv, in_=denom)
                return s_g, rt, dinv

            def stage_b(st, b0: int, gs: int, ring):
                """Finish squash scale and store the group."""
                s_g, rt, dinv = st
                scl = sqp.tile([OUT_CAPS, GROUP], F32, tag="scl", name="scl")[:, :gs]
                nc.vector.tensor_mul(out=scl, in0=rt, in1=dinv)
                # broadcast scl over d via a stride-0 AP
                scl_b = bass.AP(
                    tensor=scl.tensor, offset=scl.offset,
                    ap=[scl.ap[0], [scl.ap[1][0], gs], [0, OUT_DIM]],
                )
                v_g = sqp.tile([OUT_CAPS, GROUP, OUT_DIM], F32,
                               tag="v_g", name="v_g")[:, :gs]
                nc.vector.tensor_mul(out=v_g, in0=s_g, in1=scl_b)
                ring.dma_start(out=v[:, b0 : b0 + gs, :], in_=v_g)

            # ---- main streaming loop (stage-B pipelined one group back) ----
            pending = None
            for ti, (tb0, tb) in enumerate(DMA_PLAN):
                last_tile = ti == len(DMA_PLAN) - 1
                u_t = inp.tile([P, TILE_B, N_CHUNKS, OD], F32R,
                               tag="u_t", name="u_t")
                if not last_tile:
                    nc.sync.dma_start(
                        out=u_t[:, :tb],
                        in_=u_r[:, tb0 : tb0 + tb].bitcast(F32R),
                    )
                else:
                    # final tile: split by chunk range so the first matmuls
                    # overlap the remainder of this tile's own load
                    nc.sync.dma_start(
                        out=u_t[:, :tb, :N_SPLIT],
                        in_=u_r[:, tb0 : tb0 + tb, :N_SPLIT].bitcast(F32R),
                    )
                    nc.sync.dma_start(
                        out=u_t[:, :tb, N_SPLIT:],
                        in_=u_r[:, tb0 : tb0 + tb, N_SPLIT:].bitcast(F32R),
                    )
                g0 = 0
                while g0 < tb:
                    gs = min(GROUP, tb - g0)
                    ps = psum.tile([OUT_CAPS, GROUP, OD], F32,
                                   tag="ps", name="ps")[:, :gs]
                    matmuls(ps, u_t, g0, gs)
                    st = stage_a(ps, gs)
                    if pending is not None:
                        stage_b(*pending)
                    last_group = last_tile and g0 + gs >= tb
                    # the final store rides the SP ring, idle once u loads end
                    pending = (st, tb0 + g0,
                               gs, nc.sync if last_group else nc.scalar)
                    g0 += gs
            stage_b(*pending)

    nc.compile()
    return nc


_NC_CACHE = None


def _get_program() -> bass.Bass:
    global _NC_CACHE
    if _NC_CACHE is None:
        _NC_CACHE = _build_core_program()
    return _NC_CACHE


def kernel(u_predict: np.ndarray, b: np.ndarray, n_iterations) -> np.ndarray:
    u_predict = np.ascontiguousarray(np.asarray(u_predict, dtype=np.float32))
    b = np.ascontiguousarray(np.asarray(b, dtype=np.float32))
    nc = _get_program()
    in_maps = [
        {"u": u_predict[i * B_LOCAL : (i + 1) * B_LOCAL], "b": b}
        for i in range(N_CORES)
    ]
    results = run_bass_kernel_spmd(nc, in_maps, list(range(N_CORES))).results
    # per-core v is [OUT_CAPS, B_LOCAL, OUT_DIM] -> assemble [B, OUT, DIM]
    vs = np.stack([results[i]["v"] for i in range(N_CORES)])
    out = vs.transpose(0, 2, 1, 3).reshape(B, OUT_CAPS, OUT_DIM)
    if int(n_iterations) >= 1:
        out = out[:, None]
    return np.ascontiguousarray(out.astype(np.float32))


# revision 14
# speedup vs baseline: 1.0562x; 1.0162x over previous
"""Trainium2 Bass kernel for capsule-network AgreementRouting (n_iterations=1).

Reference computation (see problem):
    c = softmax(b, axis=-1)                  # [IN, OUT] (same for every batch)
    s[b,o,d] = sum_in c[in,o] * u[b,in,o,d]  # weighted reduce over input caps
    v = squash(s)                            # per (b,o): s * l2/(1+l2)/sqrt(l2)
    out = v[:, None]                         # [B, 1, OUT, DIM]

Strategy: data-parallel over batch across 8 NeuronCores (64 batches/core).
Per core the 47 MB u-shard is streamed through SBUF in 3-batch groups with
contiguous 5760B-per-partition DMAs; each group's in-caps reduction runs on
TensorE with softmax(b) chunks as the stationary operand
(out[o', (g,o,d)] = sum_in c[in,o'] u[in,g,o,d]), the o'==o diagonal is
extracted with a precomputed identity mask + strided reduce on VectorE, and
squash + store follow immediately per group so the post-stream tail is just
the final (2-batch) group's chain.
"""

import numpy as np

import concourse.bass as bass
import concourse.tile as tile
from concourse import bacc, mybir
from concourse.bass_utils import run_bass_kernel_spmd

F32 = mybir.dt.float32
F32R = mybir.dt.float32r

B, IN_CAPS, OUT_CAPS, OUT_DIM = 512, 1152, 10, 16
N_CORES = 8
B_LOCAL = B // N_CORES            # 64 batches per core
OD = OUT_CAPS * OUT_DIM           # 160
P = 128                           # partitions
N_CHUNKS = IN_CAPS // P           # 9 contraction chunks
GROUP = 3                         # batches per PSUM accumulation group (480 cols)

# DMA plan: 10 six-batch tiles (two 3-batch groups each) for the bulk of the
# stream, then two 2-batch tiles whose DMAs are split by chunk range so the
# final groups' matmuls overlap their own loads — keeps the post-stream tail
# down to one short dependent chain.
TILE_B = 6
DMA_PLAN = [(i * TILE_B, TILE_B) for i in range(10)] + [(60, 2), (62, 2)]
N_SPLIT = 6                       # chunk split point for the final DMA


def _build_core_program() -> bass.Bass:
    nc = bacc.Bacc(None)
    u = nc.dram_tensor("u", [B_LOCAL, IN_CAPS, OUT_CAPS, OUT_DIM], F32,
                       kind="ExternalInput")
    bp = nc.dram_tensor("b", [IN_CAPS, OUT_CAPS], F32, kind="ExternalInput")
    v = nc.dram_tensor("v", [OUT_CAPS, B_LOCAL, OUT_DIM], F32,
                       kind="ExternalOutput")

    # in-cap index mapping: in = p*N_CHUNKS + n (partition-major). Per (p, b)
    # the 9 chunk rows are contiguous in HBM -> 5760B runs per partition for
    # u and a single 360B run for b, keeping each DMA on one descriptor lane.
    u_r = u[:].rearrange("b (p n) o d -> p b n (o d)", p=P)
    b_r = bp[:].rearrange("(p n) o -> p n o", p=P)

    with tile.TileContext(nc) as tc:
        with (
            tc.tile_pool(name="singles", bufs=1) as singles,
            tc.tile_pool(name="inp", bufs=5) as inp,
            tc.tile_pool(name="psum", bufs=8, space="PSUM") as psum,
            tc.tile_pool(name="mids", bufs=4) as mids,
            tc.tile_pool(name="sq", bufs=4) as sqp,
        ):
            # ---- softmax over b rows: c[in, o] ----
            # no max-subtraction: b is all-zeros per the problem spec
            # (fill=zeros), so exp() cannot overflow
            b_sb = singles.tile([P, N_CHUNKS, OUT_CAPS], F32)
            nc.scalar.dma_start(out=b_sb, in_=b_r)
            e_sb = singles.tile([P, N_CHUNKS, OUT_CAPS], F32)
            nc.scalar.activation(
                out=e_sb, in_=b_sb,
                func=mybir.ActivationFunctionType.Exp,
            )
            esum = singles.tile([P, N_CHUNKS], F32)
            nc.vector.reduce_sum(out=esum, in_=e_sb, axis=mybir.AxisListType.X)
            einv = singles.tile([P, N_CHUNKS], F32)
            nc.vector.reciprocal(out=einv, in_=esum)
            c_sb = singles.tile([P, N_CHUNKS, OUT_CAPS], F32R)
            einv_b = bass.AP(
                tensor=einv.tensor, offset=einv.offset,
                ap=[einv.ap[0], [einv.ap[1][0], N_CHUNKS], [0, OUT_CAPS]],
            )
            nc.vector.tensor_mul(out=c_sb, in0=e_sb, in1=einv_b)

            # ---- diagonal-selection mask: mask[o', g, o, d] = (o == o') ----
            mask = singles.tile([OUT_CAPS, GROUP, OUT_CAPS, OUT_DIM], F32)
            nc.gpsimd.memset(mask, 0.0)
            nc.gpsimd.affine_select(
                out=mask, in_=mask,
                compare_op=mybir.AluOpType.not_equal,
                fill=1.0, base=0, channel_multiplier=1,
                pattern=[[0, GROUP], [-1, OUT_CAPS], [0, OUT_DIM]],
            )

            def matmuls(ps, u_t, g0: int, gs: int):
                for n in range(N_CHUNKS):
                    # float32r: fp32 bits, single-pass (tf32-like) matmul
                    nc.tensor.matmul(
                        ps,
                        c_sb[:, n, :],
                        u_t[:, g0 : g0 + gs, n, :],
                        start=(n == 0), stop=(n == N_CHUNKS - 1),
                    )

            def extract(ps, s_t, g0: int, gs: int):
                """Diagonal extract of one group into s_t[:, g0:g0+gs]."""
                masked = mids.tile(
                    [OUT_CAPS, GROUP, OUT_CAPS, OUT_DIM], F32,
                    tag="masked", name="masked")[:, :gs]
                nc.vector.tensor_mul(
                    out=masked,
                    in0=ps.rearrange("q g (o d) -> q g o d", d=OUT_DIM),
                    in1=mask[:, :gs],
                )
                nc.vector.reduce_sum(
                    out=s_t[:, g0 : g0 + gs],
                    in_=masked.rearrange("q g o d -> q g d o"),
                    axis=mybir.AxisListType.X,
                )

            def stage_a(s_t, nb: int):
                """Squash length stats for one tile (nb batches). Vector-only
                except the sqrt, which runs on the idle Scalar engine so the
                vector queue never blocks on it here."""
                s_n = s_t[:, :nb]
                sq = sqp.tile([OUT_CAPS, TILE_B, OUT_DIM], F32,
                              tag="sq", name="sq")[:, :nb]
                nc.vector.tensor_mul(out=sq, in0=s_n, in1=s_n)
                l2 = sqp.tile([OUT_CAPS, TILE_B], F32, tag="l2", name="l2")[:, :nb]
                nc.vector.reduce_sum(out=l2, in_=sq, axis=mybir.AxisListType.X)
                rt = sqp.tile([OUT_CAPS, TILE_B], F32, tag="rt", name="rt")[:, :nb]
                nc.scalar.sqrt(out=rt, in_=l2)
                denom = sqp.tile([OUT_CAPS, TILE_B], F32,
                                 tag="denom", name="denom")[:, :nb]
                nc.vector.tensor_scalar_add(out=denom, in0=l2, scalar1=1.0)
                dinv = sqp.tile([OUT_CAPS, TILE_B], F32,
                                tag="dinv", name="dinv")[:, :nb]
                nc.vector.reciprocal(out=dinv, in_=denom)
                return s_t, rt, dinv

            def stage_b(st, b0: int, nb: int, ring):
                """Finish squash scale and store the tile."""
                s_t, rt, dinv = st
                scl = sqp.tile([OUT_CAPS, TILE_B], F32, tag="scl", name="scl")[:, :nb]
                nc.vector.tensor_mul(out=scl, in0=rt, in1=dinv)
                # broadcast scl over d via a stride-0 AP
                scl_b = bass.AP(
                    tensor=scl.tensor, offset=scl.offset,
                    ap=[scl.ap[0], [scl.ap[1][0], nb], [0, OUT_DIM]],
                )
                v_t = sqp.tile([OUT_CAPS, TILE_B, OUT_DIM], F32,
                               tag="v_t", name="v_t")[:, :nb]
                nc.vector.tensor_mul(out=v_t, in0=s_t[:, :nb], in1=scl_b)
                ring.dma_start(out=v[:, b0 : b0 + nb, :], in_=v_t)

            # ---- main streaming loop (stage-B pipelined one group back) ----
            pending = None
            for ti, (tb0, tb) in enumerate(DMA_PLAN):
                last_tile = ti == len(DMA_PLAN) - 1
                u_t = inp.tile([P, TILE_B, N_CHUNKS, OD], F32R,
                               tag="u_t", name="u_t")
                if not last_tile:
                    nc.sync.dma_start(
                        out=u_t[:, :tb],
                        in_=u_r[:, tb0 : tb0 + tb].bitcast(F32R),
                    )
                else:
                    # final tile: split by chunk range so the first matmuls
                    # overlap the remainder of this tile's own load
                    nc.sync.dma_start(
                        out=u_t[:, :tb, :N_SPLIT],
                        in_=u_r[:, tb0 : tb0 + tb, :N_SPLIT].bitcast(F32R),
                    )
                    nc.sync.dma_start(
                        out=u_t[:, :tb, N_SPLIT:],
                        in_=u_r[:, tb0 : tb0 + tb, N_SPLIT:].bitcast(F32R),
                    )
                s_t = sqp.tile([OUT_CAPS, TILE_B, OUT_DIM], F32,
                               tag="s_t", name="s_t")
                g0 = 0
                while g0 < tb:
                    gs = min(GROUP, tb - g0)
                    ps = psum.tile([OUT_CAPS, GROUP, OD], F32,
                                   tag="ps", name="ps")[:, :gs]
                    matmuls(ps, u_t, g0, gs)
                    extract(ps, s_t, g0, gs)
                    g0 += gs
                st = stage_a(s_t, tb)
                if pending is not None:
                    stage_b(*pending)
                # the final store rides the SP ring, idle once u loads end
                pending = (st, tb0, tb, nc.sync if last_tile else nc.scalar)
            stage_b(*pending)

    nc.compile()
    return nc


_NC_CACHE = None


def _get_program() -> bass.Bass:
    global _NC_CACHE
    if _NC_CACHE is None:
        _NC_CACHE = _build_core_program()
    return _NC_CACHE


def kernel(u_predict: np.ndarray, b: np.ndarray, n_iterations) -> np.ndarray:
    u_predict = np.ascontiguousarray(np.asarray(u_predict, dtype=np.float32))
    b = np.ascontiguousarray(np.asarray(b, dtype=np.float32))
    nc = _get_program()
    in_maps = [
        {"u": u_predict[i * B_LOCAL : (i + 1) * B_LOCAL], "b": b}
        for i in range(N_CORES)
    ]
    results = run_bass_kernel_spmd(nc, in_maps, list(range(N_CORES))).results
    # per-core v is [OUT_CAPS, B_LOCAL, OUT_DIM] -> assemble [B, OUT, DIM]
    vs = np.stack([results[i]["v"] for i in range(N_CORES)])
    out = vs.transpose(0, 2, 1, 3).reshape(B, OUT_CAPS, OUT_DIM)
    if int(n_iterations) >= 1:
        out = out[:, None]
    return np.ascontiguousarray(out.astype(np.float32))
